# revision 32
# baseline (speedup 1.0000x reference)
"""Causal self-attention Bass/Tile kernel for TRN2, data-parallel over 8 NeuronCores.

Shapes (hardcoded): x [16, 1024, 1024] f32, W_attn [1024, 3072], b_attn [3072],
W_proj [1024, 1024], b_proj [1024].  16 heads, head dim 64.
Each core processes 2 batch elements end-to-end; no collectives.

Per-core pipeline (per batch):
  1. x -> x^T via PE transposes (fp32), evicted to fp32r tiles.
  2. q^T,k^T = (W_qk tile).T @ x^T  (transposed-output form; fp32r matmuls)
     v = (x^T tile).T @ W_v        (natural form), evicted into vext (bf16)
     with a ones-column appended per head for softmax denominators.
  3. Per head pair: scores^T = k^T.T @ q^T with K=64 row-packing of the two
     heads (tile_position), skipping fully-masked (causal) tiles; exp on
     ScalarE with the 1/8 scale folded in (no max subtraction needed: scores
     are ~N(0,1)); causal mask applied by multiplying with a precomputed
     staircase 0/1 mask; AV = vext.T @ P^T accumulated over k-tiles in PSUM,
     row 64 of the output collecting the softmax denominators; y^T scaled by
     the reciprocal denominator into fp32r tiles.
  4. out = (y^T tile).T @ W_proj + b_proj, streamed to HBM.
"""
import sys

sys.path.insert(0, "/opt/trn_rl_repo")

from contextlib import ExitStack

import numpy as np

import concourse.bass as bass
import concourse.mybir as mybir
import concourse.tile as tile
from concourse import bacc
from concourse.bass_utils import run_bass_kernel_spmd
from concourse.masks import make_identity, make_upper_triangular

F32 = mybir.dt.float32
F32R = mybir.dt.float32r
BF16 = mybir.dt.bfloat16
EXP = mybir.ActivationFunctionType.Exp

N_CORES = 8
B, T, C = 16, 1024, 1024
H, DH = 16, 64
BL = B // N_CORES          # batches per core
TT = T // 128              # token tiles (8)
KO = C // 128              # contraction chunks (8)
NQ = T // 512              # 512-wide token chunks (2)
SCALE = 1.0 / 8.0          # 1/sqrt(64)


def _emit(nc, tc, x_d, wattn_d, battn_d, wproj_d, bproj_d, out_d):
    with ExitStack() as ctx:
        const = ctx.enter_context(tc.tile_pool(name="const", bufs=1))
        xT_pool = ctx.enter_context(tc.tile_pool(name="xT", bufs=2))
        yT_pool = ctx.enter_context(tc.tile_pool(name="yT", bufs=2))
        vext_pool = ctx.enter_context(tc.tile_pool(name="vext", bufs=2))
        qk_pool = ctx.enter_context(tc.tile_pool(name="qk", bufs=3))
        pt_pool = ctx.enter_context(tc.tile_pool(name="pt", bufs=24))
        w_pool = ctx.enter_context(tc.tile_pool(name="w", bufs=14))
        rec_pool = ctx.enter_context(tc.tile_pool(name="rec", bufs=2))
        recb_pool = ctx.enter_context(tc.tile_pool(name="recb", bufs=2))
        wp_pool = ctx.enter_context(tc.tile_pool(name="wp", bufs=16))
        dram_pool = ctx.enter_context(tc.tile_pool(name="dram", bufs=2, space="DRAM"))
        psA = ctx.enter_context(tc.tile_pool(name="psA", bufs=3, space="PSUM"))
        psB = ctx.enter_context(tc.tile_pool(name="psB", bufs=3, space="PSUM"))
        psC = ctx.enter_context(tc.tile_pool(name="psC", bufs=2, space="PSUM"))

        # ---- constants ----
        ident = const.tile([128, 128], BF16)
        make_identity(nc, ident)
        # Z staircase: cols [512,640) hold upper-tri ones; slice [512-j*128, 640)
        # gives [zeros(j*128) | tril-in-(k,q)-sense] for diagonal score tiles.
        zmask = const.tile([128, 640], F32)
        nc.vector.memset(zmask, 0.0)
        make_upper_triangular(nc, zmask[:, 512:640], val=1.0, diag=True)
        ones_c = const.tile([128, 1], F32)
        nc.vector.memset(ones_c, 1.0)
        # biases
        b_qk = const.tile([128, 16], F32)
        nc.sync.dma_start(b_qk, battn_d[0 : 2 * C].rearrange("(m p) -> p m", p=128))
        brow = const.tile([1, 2 * C], F32)
        nc.sync.dma_start(brow[:, 0:C], battn_d[None, 2 * C : 3 * C])
        nc.sync.dma_start(brow[:, C : 2 * C], bproj_d[None, :])
        bv_b = const.tile([128, C], F32)
        nc.gpsimd.partition_broadcast(bv_b, brow[:, 0:C])
        bpj_b = const.tile([128, C], F32)
        nc.gpsimd.partition_broadcast(bpj_b, brow[:, C : 2 * C])

        for b in range(BL):
            # ---- phase 1: x^T ----
            xT = xT_pool.tile([128, KO, T], BF16, tag="xT")
            for tt in range(TT):
                xin = w_pool.tile([128, C], BF16, tag="w")
                nc.sync.dma_start(xin, x_d[b, tt * 128 : (tt + 1) * 128, :])
                for co in range(KO):
                    tp = psA.tile([128, 512], BF16, tag="ps")
                    nc.tensor.transpose(
                        tp[:, 0:128], xin[:, co * 128 : (co + 1) * 128], ident
                    )
                    nc.vector.tensor_copy(
                        xT[:, co, tt * 128 : (tt + 1) * 128], tp[:, 0:128]
                    )

            # ---- phase 2: v (natural layout) into vext with ones column ----
            vext = vext_pool.tile([128, TT, H, DH + 1], BF16, tag="vext")
            for nn in range(NQ):
                wv = []
                for k in range(KO):
                    wt = w_pool.tile([128, 512], BF16, tag="w")
                    nc.sync.dma_start(
                        wt,
                        wattn_d[
                            k * 128 : (k + 1) * 128,
                            2 * C + nn * 512 : 2 * C + (nn + 1) * 512,
                        ],
                    )
                    wv.append(wt)
                for m in range(TT):
                    ps = psA.tile([128, 512], F32, tag="ps")
                    for k in range(KO):
                        nc.tensor.matmul(
                            ps,
                            xT[:, k, m * 128 : (m + 1) * 128],
                            wv[k],
                            start=(k == 0),
                            stop=(k == KO - 1),
                        )
                    nc.vector.tensor_add(
                        vext[:, m, nn * 8 : (nn + 1) * 8, 0:DH],
                        ps.rearrange("p (h d) -> p h d", d=DH),
                        bv_b[:, nn * 512 : (nn + 1) * 512].rearrange(
                            "p (h d) -> p h d", d=DH
                        ),
                    )
            nc.vector.tensor_copy(
                vext[:, :, :, DH : DH + 1],
                ones_c[:, 0:1, None].to_broadcast((128, TT, H, 1)),
            )

            # ---- phase 3: per head pair: q^T/k^T, scores, softmax, AV ----
            # yT split into two tiles so phase-4's first half-K accumulation
            # only depends on head pairs 0-3 (Tile deps are tile-granular)
            yT_lo = yT_pool.tile([128, KO // 2, T], BF16, tag="yTlo")
            yT_hi = yT_pool.tile([128, KO // 2, T], BF16, tag="yThi")
            for hp in range(KO):
                yT = yT_lo if hp < KO // 2 else yT_hi
                hpo = hp % (KO // 2)
                qk = qk_pool.tile([128, 2, T], BF16, tag="qk")
                for which, mt in ((0, hp), (1, 8 + hp)):
                    wt = w_pool.tile([128, KO, 128], BF16, tag="w")
                    nc.sync.dma_start(
                        wt,
                        wattn_d[:, mt * 128 : (mt + 1) * 128].rearrange(
                            "(ko p) m -> p ko m", p=128
                        ),
                    )
                    for nn in range(NQ):
                        ps = psA.tile([128, 512], F32, tag="ps")
                        for k in range(KO):
                            nc.tensor.matmul(
                                ps,
                                wt[:, k, :],
                                xT[:, k, nn * 512 : (nn + 1) * 512],
                                start=(k == 0),
                                stop=(k == KO - 1),
                            )
                        nc.vector.tensor_add(
                            qk[:, which, nn * 512 : (nn + 1) * 512],
                            ps,
                            b_qk[:, mt : mt + 1].to_broadcast((128, 512)),
                        )

                # softmax denominators for this head pair: rows at 32-aligned
                # partitions (DVE start-partition constraint), one batched recip
                sg = rec_pool.tile([128, 512], F32, tag="sg")
                nc.vector.memset(sg, 1.0)
                def _st(kt, qc):
                    j = kt - 4 * qc
                    return 0 if j < 0 else j * 128  # first causally-valid col

                for qc in range(NQ):
                    pts = {}
                    for kt in range(4 * qc + 4):
                        j = kt - 4 * qc
                        st = _st(kt, qc)
                        st_sc = st
                        for h2 in range(2):
                            sps = psB.tile([128, 512], F32, tag="sc")
                            nc.tensor.matmul(
                                sps[:, st_sc:512],
                                qk[64 * h2 : 64 * h2 + 64, 1, kt * 128 : (kt + 1) * 128],
                                qk[
                                    64 * h2 : 64 * h2 + 64,
                                    0,
                                    qc * 512 + st_sc : (qc + 1) * 512,
                                ],
                                start=True,
                                stop=True,
                                tile_position=(64 * h2, 0),
                            )
                            pt = pt_pool.tile([128, 512], BF16, tag="pt")
                            nc.scalar.activation(
                                pt[:, st:512], sps[:, st:512], EXP, scale=SCALE
                            )
                            if j >= 0:
                                nc.vector.tensor_mul(
                                    pt[:, st : st + 128],
                                    pt[:, st : st + 128],
                                    zmask[:, 512:640],
                                )
                            pts[(h2, kt)] = (pt, st)
                    for h2 in range(2):
                        h = 2 * hp + h2
                        nkt = 4 * qc + 4
                        yps = psC.tile([128, 512], F32, tag="av")
                        for kt in range(nkt):
                            pt, st = pts[(h2, kt)]
                            nc.tensor.matmul(
                                yps[0 : DH + 1, st:512],
                                vext[:, kt, h, :],
                                pt[:, st:512],
                                start=(kt == 0),
                                stop=(kt == nkt - 1),
                            )
                        # unnormalized evict; gather this pair's denominators
                        nc.vector.tensor_copy(
                            yT[64 * h2 : 64 * h2 + 64, hpo, qc * 512 : (qc + 1) * 512],
                            yps[0:DH, :],
                        )
                        rb = (h2 * 2 + qc) * 32
                        nc.vector.tensor_copy(sg[rb : rb + 1, :], yps[DH : DH + 1, :])

                # one reciprocal for the pair, then DRAM-bounce broadcast + scale
                rec_f = recb_pool.tile([128, 512], F32, tag="recf")
                nc.vector.reciprocal(rec_f, sg)
                rec_d = dram_pool.tile([4, 512], F32, tag="recd")
                for h2 in range(2):
                    for qc in range(NQ):
                        r = h2 * 2 + qc
                        nc.sync.dma_start(
                            rec_d[r : r + 1, :], rec_f[r * 32 : r * 32 + 1, :]
                        )
                        rec_b = recb_pool.tile([128, 512], F32, tag="recb")
                        nc.sync.dma_start(
                            rec_b, rec_d[r : r + 1, :].to_broadcast((128, 512))
                        )
                        ysl = yT[64 * h2 : 64 * h2 + 64, hpo, qc * 512 : (qc + 1) * 512]
                        nc.vector.tensor_mul(
                            ysl, ysl, rec_b[64 * h2 : 64 * h2 + 64, :]
                        )

            # ---- phase 4: out = y @ W_proj + b_proj ----
            for nn in range(NQ):
                wp = []
                for k in range(KO):
                    wt = wp_pool.tile([128, 512], BF16, tag="wp")
                    nc.sync.dma_start(
                        wt, wproj_d[k * 128 : (k + 1) * 128, nn * 512 : (nn + 1) * 512]
                    )
                    wp.append(wt)
                for m in range(TT):
                    lo = psA.tile([128, 512], F32, tag="ps")
                    for k in range(KO // 2):
                        nc.tensor.matmul(
                            lo,
                            yT_lo[:, k, m * 128 : (m + 1) * 128],
                            wp[k],
                            start=(k == 0),
                            stop=(k == KO // 2 - 1),
                        )
                    osb_lo = w_pool.tile([128, 512], F32, tag="w")
                    nc.vector.tensor_add(
                        osb_lo, lo, bpj_b[:, nn * 512 : (nn + 1) * 512]
                    )
                    hi = psA.tile([128, 512], F32, tag="ps")
                    for k in range(KO // 2):
                        nc.tensor.matmul(
                            hi,
                            yT_hi[:, k, m * 128 : (m + 1) * 128],
                            wp[KO // 2 + k],
                            start=(k == 0),
                            stop=(k == KO // 2 - 1),
                        )
                    osb = w_pool.tile([128, 512], F32, tag="w")
                    nc.vector.tensor_add(osb, hi, osb_lo)
                    nc.sync.dma_start(
                        out_d[b, m * 128 : (m + 1) * 128, nn * 512 : (nn + 1) * 512],
                        osb,
                    )


_CACHE = {}


def _build():
    if "nc" in _CACHE:
        return _CACHE["nc"]
    nc = bacc.Bacc("TRN2", target_bir_lowering=False, debug=False)
    x_d = nc.dram_tensor("x", [BL, T, C], BF16, kind="ExternalInput").ap()
    wattn_d = nc.dram_tensor("W_attn", [C, 3 * C], BF16, kind="ExternalInput").ap()
    battn_d = nc.dram_tensor("b_attn", [3 * C], F32, kind="ExternalInput").ap()
    wproj_d = nc.dram_tensor("W_proj", [C, C], BF16, kind="ExternalInput").ap()
    bproj_d = nc.dram_tensor("b_proj", [C], F32, kind="ExternalInput").ap()
    out_d = nc.dram_tensor("out", [BL, T, C], F32, kind="ExternalOutput").ap()
    with tile.TileContext(nc) as tc:
        _emit(nc, tc, x_d, wattn_d, battn_d, wproj_d, bproj_d, out_d)
    nc.compile()
    _CACHE["nc"] = nc
    return nc


def kernel(x, W_attn, b_attn, W_proj, b_proj, _trace=False):
    nc = _build()
    import ml_dtypes

    x = np.ascontiguousarray(np.asarray(x, dtype=np.float32).astype(ml_dtypes.bfloat16))
    W_attn = np.ascontiguousarray(np.asarray(W_attn, dtype=np.float32).astype(ml_dtypes.bfloat16))
    b_attn = np.ascontiguousarray(np.asarray(b_attn, dtype=np.float32))
    W_proj = np.ascontiguousarray(np.asarray(W_proj, dtype=np.float32).astype(ml_dtypes.bfloat16))
    b_proj = np.ascontiguousarray(np.asarray(b_proj, dtype=np.float32))
    in_maps = [
        {
            "x": x[i * BL : (i + 1) * BL],
            "W_attn": W_attn,
            "b_attn": b_attn,
            "W_proj": W_proj,
            "b_proj": b_proj,
        }
        for i in range(N_CORES)
    ]
    res = run_bass_kernel_spmd(nc, in_maps, core_ids=list(range(N_CORES)), trace=_trace)
    out = np.concatenate([res.results[i]["out"] for i in range(N_CORES)], axis=0)
    if _trace:
        kernel.last_results = res
    return out


# revision 34
# speedup vs baseline: 1.0117x; 1.0117x over previous
"""Causal self-attention Bass/Tile kernel for TRN2, data-parallel over 8 NeuronCores.

Shapes (hardcoded): x [16, 1024, 1024] f32, W_attn [1024, 3072], b_attn [3072],
W_proj [1024, 1024], b_proj [1024].  16 heads, head dim 64.
Each core processes 2 batch elements end-to-end; no collectives.

Per-core pipeline (per batch):
  1. x -> x^T via PE transposes (fp32), evicted to fp32r tiles.
  2. q^T,k^T = (W_qk tile).T @ x^T  (transposed-output form; fp32r matmuls)
     v = (x^T tile).T @ W_v        (natural form), evicted into vext (bf16)
     with a ones-column appended per head for softmax denominators.
  3. Per head pair: scores^T = k^T.T @ q^T with K=64 row-packing of the two
     heads (tile_position), skipping fully-masked (causal) tiles; exp on
     ScalarE with the 1/8 scale folded in (no max subtraction needed: scores
     are ~N(0,1)); causal mask applied by multiplying with a precomputed
     staircase 0/1 mask; AV = vext.T @ P^T accumulated over k-tiles in PSUM,
     row 64 of the output collecting the softmax denominators; y^T scaled by
     the reciprocal denominator into fp32r tiles.
  4. out = (y^T tile).T @ W_proj + b_proj, streamed to HBM.
"""
import sys

sys.path.insert(0, "/opt/trn_rl_repo")

from contextlib import ExitStack

import numpy as np

import concourse.bass as bass
import concourse.mybir as mybir
import concourse.tile as tile
from concourse import bacc
from concourse.bass_utils import run_bass_kernel_spmd
from concourse.masks import make_identity, make_upper_triangular

F32 = mybir.dt.float32
F32R = mybir.dt.float32r
BF16 = mybir.dt.bfloat16
EXP = mybir.ActivationFunctionType.Exp

N_CORES = 8
B, T, C = 16, 1024, 1024
H, DH = 16, 64
BL = B // N_CORES          # batches per core
TT = T // 128              # token tiles (8)
KO = C // 128              # contraction chunks (8)
NQ = T // 512              # 512-wide token chunks (2)
SCALE = 1.0 / 8.0          # 1/sqrt(64)


def _emit(nc, tc, x_d, wattn_d, battn_d, wproj_d, bproj_d, out_d):
    with ExitStack() as ctx:
        const = ctx.enter_context(tc.tile_pool(name="const", bufs=1))
        xT_pool = ctx.enter_context(tc.tile_pool(name="xT", bufs=2))
        yT_pool = ctx.enter_context(tc.tile_pool(name="yT", bufs=2))
        vext_pool = ctx.enter_context(tc.tile_pool(name="vext", bufs=2))
        qk_pool = ctx.enter_context(tc.tile_pool(name="qk", bufs=3))
        pt_pool = ctx.enter_context(tc.tile_pool(name="pt", bufs=24))
        w_pool = ctx.enter_context(tc.tile_pool(name="w", bufs=14))
        rec_pool = ctx.enter_context(tc.tile_pool(name="rec", bufs=2))
        recb_pool = ctx.enter_context(tc.tile_pool(name="recb", bufs=2))
        wp_pool = ctx.enter_context(tc.tile_pool(name="wp", bufs=16))
        dram_pool = ctx.enter_context(tc.tile_pool(name="dram", bufs=2, space="DRAM"))
        psA = ctx.enter_context(tc.tile_pool(name="psA", bufs=3, space="PSUM"))
        psB = ctx.enter_context(tc.tile_pool(name="psB", bufs=3, space="PSUM"))
        psC = ctx.enter_context(tc.tile_pool(name="psC", bufs=2, space="PSUM"))

        # ---- constants ----
        ident = const.tile([128, 128], BF16)
        make_identity(nc, ident)
        # Z staircase: cols [512,640) hold upper-tri ones; slice [512-j*128, 640)
        # gives [zeros(j*128) | tril-in-(k,q)-sense] for diagonal score tiles.
        zmask = const.tile([128, 640], F32)
        nc.vector.memset(zmask, 0.0)
        make_upper_triangular(nc, zmask[:, 512:640], val=1.0, diag=True)
        ones_c = const.tile([128, 1], F32)
        nc.vector.memset(ones_c, 1.0)
        # biases
        b_qk = const.tile([128, 16], F32)
        nc.sync.dma_start(b_qk, battn_d[0 : 2 * C].rearrange("(m p) -> p m", p=128))
        brow = const.tile([1, 2 * C], F32)
        nc.sync.dma_start(brow[:, 0:C], battn_d[None, 2 * C : 3 * C])
        nc.sync.dma_start(brow[:, C : 2 * C], bproj_d[None, :])
        bv_b = const.tile([128, C], F32)
        nc.gpsimd.partition_broadcast(bv_b, brow[:, 0:C])
        bpj_b = const.tile([128, C], F32)
        nc.gpsimd.partition_broadcast(bpj_b, brow[:, C : 2 * C])

        for b in range(BL):
            # ---- phase 1: x^T ----
            xT = xT_pool.tile([128, KO, T], BF16, tag="xT")
            for tt in range(TT):
                xin = w_pool.tile([128, C], BF16, tag="w")
                nc.sync.dma_start(xin, x_d[b, tt * 128 : (tt + 1) * 128, :])
                for co in range(KO):
                    tp = psA.tile([128, 512], BF16, tag="ps")
                    nc.tensor.transpose(
                        tp[:, 0:128], xin[:, co * 128 : (co + 1) * 128], ident
                    )
                    nc.vector.tensor_copy(
                        xT[:, co, tt * 128 : (tt + 1) * 128], tp[:, 0:128]
                    )

            # ---- phase 2: v (natural layout) into vext with ones column ----
            vext = vext_pool.tile([128, TT, H, DH + 1], BF16, tag="vext")
            for nn in range(NQ):
                wv = []
                for k in range(KO):
                    wt = w_pool.tile([128, 512], BF16, tag="w")
                    nc.sync.dma_start(
                        wt,
                        wattn_d[
                            k * 128 : (k + 1) * 128,
                            2 * C + nn * 512 : 2 * C + (nn + 1) * 512,
                        ],
                    )
                    wv.append(wt)
                for m in range(TT):
                    ps = psA.tile([128, 512], F32, tag="ps")
                    for k in range(KO):
                        nc.tensor.matmul(
                            ps,
                            xT[:, k, m * 128 : (m + 1) * 128],
                            wv[k],
                            start=(k == 0),
                            stop=(k == KO - 1),
                        )
                    nc.vector.tensor_add(
                        vext[:, m, nn * 8 : (nn + 1) * 8, 0:DH],
                        ps.rearrange("p (h d) -> p h d", d=DH),
                        bv_b[:, nn * 512 : (nn + 1) * 512].rearrange(
                            "p (h d) -> p h d", d=DH
                        ),
                    )
            nc.vector.tensor_copy(
                vext[:, :, :, DH : DH + 1],
                ones_c[:, 0:1, None].to_broadcast((128, TT, H, 1)),
            )

            # preload W_proj so phase-4 DMAs aren't queued behind attention DMAs
            wp_all = []
            for nn in range(NQ):
                wpn = []
                for k in range(KO):
                    wt = wp_pool.tile([128, 512], BF16, tag="wp")
                    nc.sync.dma_start(
                        wt, wproj_d[k * 128 : (k + 1) * 128, nn * 512 : (nn + 1) * 512]
                    )
                    wpn.append(wt)
                wp_all.append(wpn)

            # ---- phase 3: per head pair: q^T/k^T, scores, softmax, AV ----
            # yT split into two tiles so phase-4's first half-K accumulation
            # only depends on head pairs 0-3 (Tile deps are tile-granular)
            yT_lo = yT_pool.tile([128, KO // 2, T], BF16, tag="yTlo")
            yT_hi = yT_pool.tile([128, KO // 2, T], BF16, tag="yThi")
            for hp in range(KO):
                yT = yT_lo if hp < KO // 2 else yT_hi
                hpo = hp % (KO // 2)
                qk = qk_pool.tile([128, 2, T], BF16, tag="qk")
                for which, mt in ((0, hp), (1, 8 + hp)):
                    wt = w_pool.tile([128, KO, 128], BF16, tag="w")
                    nc.sync.dma_start(
                        wt,
                        wattn_d[:, mt * 128 : (mt + 1) * 128].rearrange(
                            "(ko p) m -> p ko m", p=128
                        ),
                    )
                    for nn in range(NQ):
                        ps = psA.tile([128, 512], F32, tag="ps")
                        for k in range(KO):
                            nc.tensor.matmul(
                                ps,
                                wt[:, k, :],
                                xT[:, k, nn * 512 : (nn + 1) * 512],
                                start=(k == 0),
                                stop=(k == KO - 1),
                            )
                        nc.vector.tensor_add(
                            qk[:, which, nn * 512 : (nn + 1) * 512],
                            ps,
                            b_qk[:, mt : mt + 1].to_broadcast((128, 512)),
                        )

                # softmax denominators for this head pair: rows at 32-aligned
                # partitions (DVE start-partition constraint), one batched recip
                sg = rec_pool.tile([128, 512], F32, tag="sg")
                nc.vector.memset(sg, 1.0)
                def _st(kt, qc):
                    j = kt - 4 * qc
                    return 0 if j < 0 else j * 128  # first causally-valid col

                for qc in range(NQ):
                    pts = {}
                    for kt in range(4 * qc + 4):
                        j = kt - 4 * qc
                        st = _st(kt, qc)
                        st_sc = st
                        for h2 in range(2):
                            sps = psB.tile([128, 512], F32, tag="sc")
                            nc.tensor.matmul(
                                sps[:, st_sc:512],
                                qk[64 * h2 : 64 * h2 + 64, 1, kt * 128 : (kt + 1) * 128],
                                qk[
                                    64 * h2 : 64 * h2 + 64,
                                    0,
                                    qc * 512 + st_sc : (qc + 1) * 512,
                                ],
                                start=True,
                                stop=True,
                                tile_position=(64 * h2, 0),
                            )
                            pt = pt_pool.tile([128, 512], BF16, tag="pt")
                            nc.scalar.activation(
                                pt[:, st:512], sps[:, st:512], EXP, scale=SCALE
                            )
                            if j >= 0:
                                nc.vector.tensor_mul(
                                    pt[:, st : st + 128],
                                    pt[:, st : st + 128],
                                    zmask[:, 512:640],
                                )
                            pts[(h2, kt)] = (pt, st)
                    for h2 in range(2):
                        h = 2 * hp + h2
                        nkt = 4 * qc + 4
                        yps = psC.tile([128, 512], F32, tag="av")
                        for kt in range(nkt):
                            pt, st = pts[(h2, kt)]
                            nc.tensor.matmul(
                                yps[0 : DH + 1, st:512],
                                vext[:, kt, h, :],
                                pt[:, st:512],
                                start=(kt == 0),
                                stop=(kt == nkt - 1),
                            )
                        # unnormalized evict; gather this pair's denominators
                        nc.vector.tensor_copy(
                            yT[64 * h2 : 64 * h2 + 64, hpo, qc * 512 : (qc + 1) * 512],
                            yps[0:DH, :],
                        )
                        rb = (h2 * 2 + qc) * 32
                        nc.vector.tensor_copy(sg[rb : rb + 1, :], yps[DH : DH + 1, :])

                # one reciprocal for the pair, then DRAM-bounce broadcast + scale
                rec_f = recb_pool.tile([128, 512], F32, tag="recf")
                nc.vector.reciprocal(rec_f, sg)
                rec_d = dram_pool.tile([4, 512], F32, tag="recd")
                for h2 in range(2):
                    for qc in range(NQ):
                        r = h2 * 2 + qc
                        nc.sync.dma_start(
                            rec_d[r : r + 1, :], rec_f[r * 32 : r * 32 + 1, :]
                        )
                        rec_b = recb_pool.tile([128, 512], F32, tag="recb")
                        nc.sync.dma_start(
                            rec_b, rec_d[r : r + 1, :].to_broadcast((128, 512))
                        )
                        ysl = yT[64 * h2 : 64 * h2 + 64, hpo, qc * 512 : (qc + 1) * 512]
                        nc.vector.tensor_mul(
                            ysl, ysl, rec_b[64 * h2 : 64 * h2 + 64, :]
                        )

            # ---- phase 4: out = y @ W_proj + b_proj ----
            for nn in range(NQ):
                wp = wp_all[nn]
                for m in range(TT):
                    lo = psA.tile([128, 512], F32, tag="ps")
                    for k in range(KO // 2):
                        nc.tensor.matmul(
                            lo,
                            yT_lo[:, k, m * 128 : (m + 1) * 128],
                            wp[k],
                            start=(k == 0),
                            stop=(k == KO // 2 - 1),
                        )
                    osb_lo = w_pool.tile([128, 512], F32, tag="w")
                    nc.vector.tensor_add(
                        osb_lo, lo, bpj_b[:, nn * 512 : (nn + 1) * 512]
                    )
                    hi = psA.tile([128, 512], F32, tag="ps")
                    for k in range(KO // 2):
                        nc.tensor.matmul(
                            hi,
                            yT_hi[:, k, m * 128 : (m + 1) * 128],
                            wp[KO // 2 + k],
                            start=(k == 0),
                            stop=(k == KO // 2 - 1),
                        )
                    osb = w_pool.tile([128, 512], F32, tag="w")
                    nc.vector.tensor_add(osb, hi, osb_lo)
                    nc.sync.dma_start(
                        out_d[b, m * 128 : (m + 1) * 128, nn * 512 : (nn + 1) * 512],
                        osb,
                    )


_CACHE = {}


def _build():
    if "nc" in _CACHE:
        return _CACHE["nc"]
    nc = bacc.Bacc("TRN2", target_bir_lowering=False, debug=False)
    x_d = nc.dram_tensor("x", [BL, T, C], BF16, kind="ExternalInput").ap()
    wattn_d = nc.dram_tensor("W_attn", [C, 3 * C], BF16, kind="ExternalInput").ap()
    battn_d = nc.dram_tensor("b_attn", [3 * C], F32, kind="ExternalInput").ap()
    wproj_d = nc.dram_tensor("W_proj", [C, C], BF16, kind="ExternalInput").ap()
    bproj_d = nc.dram_tensor("b_proj", [C], F32, kind="ExternalInput").ap()
    out_d = nc.dram_tensor("out", [BL, T, C], F32, kind="ExternalOutput").ap()
    with tile.TileContext(nc) as tc:
        _emit(nc, tc, x_d, wattn_d, battn_d, wproj_d, bproj_d, out_d)
    nc.compile()
    _CACHE["nc"] = nc
    return nc


def kernel(x, W_attn, b_attn, W_proj, b_proj, _trace=False):
    nc = _build()
    import ml_dtypes

    x = np.ascontiguousarray(np.asarray(x, dtype=np.float32).astype(ml_dtypes.bfloat16))
    W_attn = np.ascontiguousarray(np.asarray(W_attn, dtype=np.float32).astype(ml_dtypes.bfloat16))
    b_attn = np.ascontiguousarray(np.asarray(b_attn, dtype=np.float32))
    W_proj = np.ascontiguousarray(np.asarray(W_proj, dtype=np.float32).astype(ml_dtypes.bfloat16))
    b_proj = np.ascontiguousarray(np.asarray(b_proj, dtype=np.float32))
    in_maps = [
        {
            "x": x[i * BL : (i + 1) * BL],
            "W_attn": W_attn,
            "b_attn": b_attn,
            "W_proj": W_proj,
            "b_proj": b_proj,
        }
        for i in range(N_CORES)
    ]
    res = run_bass_kernel_spmd(nc, in_maps, core_ids=list(range(N_CORES)), trace=_trace)
    out = np.concatenate([res.results[i]["out"] for i in range(N_CORES)], axis=0)
    if _trace:
        kernel.last_results = res
    return out
